# revision 2
# baseline (speedup 1.0000x reference)
"""DecoderRNN (single-step GRU + Bahdanau attention + vocab projection) on 8 TRN2 cores.

Sharding:
  - GRU: hidden dim row-sharded (each core owns a 128-slice of H for all 3 gates),
    h_new combined via AllGather.
  - Attention score vector v2 = w_score^T @ atten_W[:, H:2H]: column-sharded
    (128 output cols per core), combined in the same AllGather round.
    Note softmax(scores + c) == softmax(scores) for any constant c, so the
    hidden-state contribution to the (pre-tanh-free) attention scores cancels;
    only enc @ v2 matters.
  - encoder_outputs: sequence-sharded (512 rows per core); local softmax stats +
    partial context AllGathered and combined on every core.
  - out_W/out_b: vocab-sharded (6400 padded rows per core); log-softmax stats
    AllGathered for the global logsumexp.
"""

import numpy as np

import concourse.bass as bass
import concourse.mybir as mybir
import concourse.tile as tile
from concourse import bacc, bass_isa
from concourse.bass_utils import run_bass_kernel_spmd
from concourse.masks import make_identity

H = 1024
H2 = 2048
V = 50257
S = 4096
NC = 8
VS = 6400        # padded vocab rows per core (8*6400 = 51200 >= V)
VT = VS // 128   # 50 vocab tiles per core
SS = S // NC     # 512 encoder rows per core
ST = SS // 128   # 4 encoder tiles per core

F32 = mybir.dt.float32
I32 = mybir.dt.int32
AX = mybir.AxisListType
ALU = mybir.AluOpType
AF = mybir.ActivationFunctionType

NEG_BIG = -1.0e30


def build_nc():
    nc = bacc.Bacc(None, target_bir_lowering=False, num_devices=NC)

    # ---- per-core external inputs ----
    tok = nc.dram_tensor("tok", [1, 1], I32, kind="ExternalInput")
    emb = nc.dram_tensor("emb", [V, H], F32, kind="ExternalInput")
    lc = nc.dram_tensor("lc", [1, H], F32, kind="ExternalInput")
    lh = nc.dram_tensor("lh", [1, H], F32, kind="ExternalInput")
    lh_col = nc.dram_tensor("lh_col", [128, 1], F32, kind="ExternalInput")
    enc = nc.dram_tensor("enc", [SS, H], F32, kind="ExternalInput")
    w_ihx = nc.dram_tensor("w_ihx", [3 * 128, H2], F32, kind="ExternalInput")
    w_hhx = nc.dram_tensor("w_hhx", [3 * 128, H], F32, kind="ExternalInput")
    b_ihx = nc.dram_tensor("b_ihx", [128, 3], F32, kind="ExternalInput")
    b_hhx = nc.dram_tensor("b_hhx", [128, 3], F32, kind="ExternalInput")
    ws_t = nc.dram_tensor("ws_t", [128, 8], F32, kind="ExternalInput")
    awx = nc.dram_tensor("awx", [H, 128], F32, kind="ExternalInput")
    outWx = nc.dram_tensor("outWx", [VS, H2], F32, kind="ExternalInput")
    outbx = nc.dram_tensor("outbx", [128, VT], F32, kind="ExternalInput")

    # ---- per-core external outputs ----
    out_logits = nc.dram_tensor("out_logits", [128, VT], F32, kind="ExternalOutput")
    out_w = nc.dram_tensor("out_w", [128, ST], F32, kind="ExternalOutput")
    out_context = nc.dram_tensor("out_context", [1, H], F32, kind="ExternalOutput")
    out_hidden = nc.dram_tensor("out_hidden", [1, H], F32, kind="ExternalOutput")

    rg = [list(range(NC))]

    with tile.TileContext(nc) as tc:
        with (
            tc.tile_pool(name="const", bufs=1) as cp,
            tc.tile_pool(name="wstream", bufs=10) as wp,
            tc.tile_pool(name="scratch", bufs=2) as sp,
            tc.tile_pool(name="psum", bufs=1, space="PSUM") as pp,
            tc.tile_pool(name="dram", bufs=1, space="DRAM") as dp,
        ):
            # ---------- offsets for replicating gathers ----------
            tok_sb = cp.tile([1, 1], I32)
            nc.sync.dma_start(tok_sb[:], tok[:])
            tok_col = cp.tile([128, 1], I32)
            nc.gpsimd.partition_broadcast(tok_col[:], tok_sb[:])
            zero_col = cp.tile([128, 1], I32)
            nc.vector.memset(zero_col[:], 0)

            # ---------- broadcast-replicated vectors ----------
            # x_rep = [embedded ; last_context] replicated on all 128 partitions
            x_rep = cp.tile([128, H2], F32)
            nc.gpsimd.indirect_dma_start(
                out=x_rep[:, 0:H], out_offset=None,
                in_=emb[:],
                in_offset=bass.IndirectOffsetOnAxis(ap=tok_col[:], axis=0),
            )
            nc.gpsimd.indirect_dma_start(
                out=x_rep[:, H:H2], out_offset=None,
                in_=lc[:],
                in_offset=bass.IndirectOffsetOnAxis(ap=zero_col[:], axis=0),
            )
            h_rep = cp.tile([128, H], F32)
            nc.gpsimd.indirect_dma_start(
                out=h_rep[:], out_offset=None,
                in_=lh[:],
                in_offset=bass.IndirectOffsetOnAxis(ap=zero_col[:], axis=0),
            )

            # ---------- GRU gate mat-vecs (row shard: 128 rows of H per gate) ----------
            wih_t = cp.tile([128, 3, H2], F32)
            nc.sync.dma_start(wih_t[:], w_ihx[:].rearrange("(g p) k -> p g k", p=128))
            whh_t = cp.tile([128, 3, H], F32)
            nc.sync.dma_start(whh_t[:], w_hhx[:].rearrange("(g p) k -> p g k", p=128))
            bih_t = cp.tile([128, 3], F32)
            nc.sync.dma_start(bih_t[:], b_ihx[:])
            bhh_t = cp.tile([128, 3], F32)
            nc.sync.dma_start(bhh_t[:], b_hhx[:])

            gi = cp.tile([128, 3], F32)
            gh = cp.tile([128, 3], F32)
            for g in range(3):
                sc2 = sp.tile([128, H2], F32, tag="sc2")
                nc.vector.tensor_mul(out=sc2[:], in0=wih_t[:, g], in1=x_rep[:])
                nc.scalar.activation(sc2[:], sc2[:], AF.Copy,
                                     accum_out=gi[:, g : g + 1])
            for g in range(3):
                sc1 = sp.tile([128, H], F32, tag="sc1")
                nc.vector.tensor_mul(out=sc1[:], in0=whh_t[:, g], in1=h_rep[:])
                nc.scalar.activation(sc1[:], sc1[:], AF.Copy,
                                     accum_out=gh[:, g : g + 1])
            nc.vector.tensor_add(out=gi[:], in0=gi[:], in1=bih_t[:])
            nc.vector.tensor_add(out=gh[:], in0=gh[:], in1=bhh_t[:])

            # ---------- gates ----------
            rz = cp.tile([128, 2], F32)
            nc.vector.tensor_add(out=rz[:], in0=gi[:, 0:2], in1=gh[:, 0:2])
            nc.scalar.activation(rz[:], rz[:], AF.Sigmoid)
            n_t = cp.tile([128, 1], F32)
            nc.vector.tensor_mul(out=n_t[:], in0=rz[:, 0:1], in1=gh[:, 2:3])
            nc.vector.tensor_add(out=n_t[:], in0=n_t[:], in1=gi[:, 2:3])
            nc.scalar.activation(n_t[:], n_t[:], AF.Tanh)
            h_col = cp.tile([128, 1], F32)
            nc.sync.dma_start(h_col[:], lh_col[:])
            hnew_col = cp.tile([128, 1], F32)
            nc.vector.tensor_sub(out=hnew_col[:], in0=h_col[:], in1=n_t[:])
            nc.vector.tensor_mul(out=hnew_col[:], in0=hnew_col[:], in1=rz[:, 1:2])
            nc.vector.tensor_add(out=hnew_col[:], in0=hnew_col[:], in1=n_t[:])

            # ---------- v2 column shard: v2_slice = w_score^T @ atten_W[:, Hc..Hc+128] ----------
            aw_t = cp.tile([128, 8, 128], F32)
            nc.sync.dma_start(aw_t[:], awx[:].rearrange("(o p) n -> p o n", p=128))
            ws_sb = cp.tile([128, 8], F32)
            nc.sync.dma_start(ws_sb[:], ws_t[:])
            ps_v = pp.tile([1, 128], F32, name="ps_v")
            for o in range(8):
                nc.tensor.matmul(
                    out=ps_v[:], lhsT=ws_sb[:, o : o + 1], rhs=aw_t[:, o],
                    start=(o == 0), stop=(o == 7),
                )
            v2row = cp.tile([1, 128], F32)
            nc.scalar.copy(v2row[:], ps_v[:])

            # ---------- h_new to row layout via PE transpose ----------
            ident = cp.tile([128, 128], F32)
            make_identity(nc, ident[:])
            ps_h = pp.tile([1, 128], F32, name="ps_h")
            nc.tensor.matmul(out=ps_h[:], lhsT=hnew_col[:], rhs=ident[:], start=True, stop=True)
            hrow = cp.tile([1, 128], F32)
            nc.scalar.copy(hrow[:], ps_h[:])

            # ---------- AllGather h_new and v2 ----------
            agh_in = dp.tile([1, 128], F32)
            agh_out = dp.tile([NC, 128], F32)
            nc.gpsimd.dma_start(agh_in[:], hrow[:])
            nc.gpsimd.collective_compute(
                "AllGather", ALU.bypass, replica_groups=rg,
                ins=[agh_in.opt()], outs=[agh_out.opt()],
            )
            agv_in = dp.tile([1, 128], F32)
            agv_out = dp.tile([NC, 128], F32)
            nc.gpsimd.dma_start(agv_in[:], v2row[:])
            nc.gpsimd.collective_compute(
                "AllGather", ALU.bypass, replica_groups=rg,
                ins=[agv_in.opt()], outs=[agv_out.opt()],
            )

            # out_hidden (bounce via SBUF)
            hfull_row = cp.tile([1, H], F32)
            nc.sync.dma_start(hfull_row[:], agh_out[:].rearrange("a b -> (a b)")[None, :])
            nc.sync.dma_start(out_hidden[:], hfull_row[:])

            # replicate h_new and v2 across partitions
            xout_rep = cp.tile([128, H2], F32)
            nc.gpsimd.indirect_dma_start(
                out=xout_rep[:, 0:H], out_offset=None,
                in_=agh_out[:].rearrange("a b -> (a b)")[None, :],
                in_offset=bass.IndirectOffsetOnAxis(ap=zero_col[:], axis=0),
            )
            v2_rep = cp.tile([128, H], F32)
            nc.gpsimd.indirect_dma_start(
                out=v2_rep[:], out_offset=None,
                in_=agv_out[:].rearrange("a b -> (a b)")[None, :],
                in_offset=bass.IndirectOffsetOnAxis(ap=zero_col[:], axis=0),
            )

            # ---------- attention scores + local softmax + partial context ----------
            enc_t = cp.tile([128, ST, H], F32)
            nc.sync.dma_start(enc_t[:], enc[:].rearrange("(t p) k -> p t k", p=128))
            sc = cp.tile([128, ST], F32)
            for t in range(ST):
                sc1 = sp.tile([128, H], F32, tag="sc1")
                nc.vector.tensor_mul(out=sc1[:], in0=enc_t[:, t], in1=v2_rep[:])
                nc.scalar.activation(sc1[:], sc1[:], AF.Copy,
                                     accum_out=sc[:, t : t + 1])
            pmax = cp.tile([128, 1], F32)
            nc.vector.reduce_max(pmax[:], sc[:], axis=AX.X)
            m_loc = cp.tile([128, 1], F32)
            nc.gpsimd.partition_all_reduce(m_loc[:], pmax[:], channels=128,
                                           reduce_op=bass_isa.ReduceOp.max)
            negm = cp.tile([128, 1], F32)
            nc.vector.tensor_scalar_mul(negm[:], m_loc[:], -1.0)
            e_t = cp.tile([128, ST], F32)
            psum_part = cp.tile([128, 1], F32)
            nc.scalar.activation(e_t[:], sc[:], AF.Exp, bias=negm[:], scale=1.0,
                                 accum_out=psum_part[:])
            s_loc = cp.tile([128, 1], F32)
            nc.gpsimd.partition_all_reduce(s_loc[:], psum_part[:], channels=128,
                                           reduce_op=bass_isa.ReduceOp.add)
            ps_pa = pp.tile([1, 512], F32, name="ps_pa")
            ps_pb = pp.tile([1, 512], F32, name="ps_pb")
            for t in range(ST):
                nc.tensor.matmul(out=ps_pa[:], lhsT=e_t[:, t : t + 1],
                                 rhs=enc_t[:, t, 0:512], start=(t == 0), stop=(t == ST - 1))
            for t in range(ST):
                nc.tensor.matmul(out=ps_pb[:], lhsT=e_t[:, t : t + 1],
                                 rhs=enc_t[:, t, 512:H], start=(t == 0), stop=(t == ST - 1))

            # ---------- AllGather [m_loc, s_loc, pctx] ----------
            ag2_row = cp.tile([1, 2 + H], F32)
            nc.scalar.copy(ag2_row[:, 0:1], m_loc[0:1, :])
            nc.scalar.copy(ag2_row[:, 1:2], s_loc[0:1, :])
            nc.scalar.copy(ag2_row[:, 2 : 2 + 512], ps_pa[:])
            nc.scalar.copy(ag2_row[:, 2 + 512 : 2 + H], ps_pb[:])
            ag2_in = dp.tile([1, 2 + H], F32)
            ag2_out = dp.tile([NC, 2 + H], F32)
            nc.gpsimd.dma_start(ag2_in[:], ag2_row[:])
            nc.gpsimd.collective_compute(
                "AllGather", ALU.bypass, replica_groups=rg,
                ins=[ag2_in.opt()], outs=[ag2_out.opt()],
            )

            # ---------- combine: global softmax + context ----------
            mrow = cp.tile([1, NC], F32)
            nc.sync.dma_start(mrow[:], ag2_out[:, 0:1].rearrange("a b -> (a b)")[None, :])
            srow = cp.tile([1, NC], F32)
            nc.sync.dma_start(srow[:], ag2_out[:, 1:2].rearrange("a b -> (a b)")[None, :])
            pctx_t = cp.tile([NC, H], F32)
            nc.sync.dma_start(pctx_t[:], ag2_out[:, 2 : 2 + H])

            gM = cp.tile([1, 1], F32)
            nc.vector.reduce_max(gM[:], mrow[:], axis=AX.X)
            dm = cp.tile([1, NC], F32)
            nc.vector.tensor_scalar(out=dm[:], in0=mrow[:], scalar1=gM[:],
                                    scalar2=None, op0=ALU.subtract)
            al = cp.tile([1, NC], F32)
            nc.scalar.activation(al[:], dm[:], AF.Exp)
            wsum = cp.tile([1, NC], F32)
            nc.vector.tensor_mul(out=wsum[:], in0=al[:], in1=srow[:])
            S_tot = cp.tile([1, 1], F32)
            nc.vector.reduce_sum(S_tot[:], wsum[:], axis=AX.X)
            rS = cp.tile([1, 1], F32)
            nc.vector.reciprocal(rS[:], S_tot[:])
            grow = cp.tile([1, NC], F32)
            nc.vector.tensor_scalar_mul(grow[:], al[:], rS[:])

            ones11 = cp.tile([1, 1], F32)
            nc.vector.memset(ones11[:], 1.0)
            ps_g = pp.tile([NC, 1], F32, name="ps_g")
            nc.tensor.matmul(out=ps_g[:], lhsT=grow[:], rhs=ones11[:], start=True, stop=True)
            gcol = cp.tile([NC, 1], F32)
            nc.scalar.copy(gcol[:], ps_g[:])

            ps_ca = pp.tile([1, 512], F32, name="ps_ca")
            ps_cb = pp.tile([1, 512], F32, name="ps_cb")
            nc.tensor.matmul(out=ps_ca[:], lhsT=gcol[:], rhs=pctx_t[:, 0:512], start=True, stop=True)
            nc.tensor.matmul(out=ps_cb[:], lhsT=gcol[:], rhs=pctx_t[:, 512:H], start=True, stop=True)
            ctx_row = cp.tile([1, H], F32)
            nc.scalar.copy(ctx_row[:, 0:512], ps_ca[:])
            nc.scalar.copy(ctx_row[:, 512:H], ps_cb[:])
            nc.sync.dma_start(out_context[:], ctx_row[:])

            # attention weights output: e_t * (exp(m_loc - M) / S_tot)
            dmine = cp.tile([1, 1], F32)
            nc.vector.tensor_sub(out=dmine[:], in0=m_loc[0:1, :], in1=gM[:])
            nc.scalar.activation(dmine[:], dmine[:], AF.Exp)
            nc.vector.tensor_mul(out=dmine[:], in0=dmine[:], in1=rS[:])
            gm_col = cp.tile([128, 1], F32)
            nc.gpsimd.partition_broadcast(gm_col[:], dmine[:])
            w_tile = cp.tile([128, ST], F32)
            nc.vector.tensor_scalar_mul(w_tile[:], e_t[:], gm_col[:])
            nc.sync.dma_start(out_w[:], w_tile[:])

            # replicate context into xout_rep
            nc.gpsimd.partition_broadcast(xout_rep[:, H:H2], ctx_row[:])

            # ---------- vocab shard logits ----------
            outb_t = cp.tile([128, VT], F32)
            nc.sync.dma_start(outb_t[:], outbx[:])
            lg = cp.tile([128, VT], F32)
            for i in range(VT):
                wt = wp.tile([128, H2], F32, tag="wt")
                nc.sync.dma_start(wt[:], outWx[128 * i : 128 * (i + 1), :])
                sc2 = sp.tile([128, H2], F32, tag="sc2")
                nc.vector.tensor_mul(out=sc2[:], in0=wt[:], in1=xout_rep[:])
                nc.scalar.activation(sc2[:], sc2[:], AF.Copy,
                                     accum_out=lg[:, i : i + 1])
            nc.vector.tensor_add(out=lg[:], in0=lg[:], in1=outb_t[:])

            # ---------- log-softmax stats ----------
            pm2 = cp.tile([128, 1], F32)
            nc.vector.reduce_max(pm2[:], lg[:], axis=AX.X)
            m2 = cp.tile([128, 1], F32)
            nc.gpsimd.partition_all_reduce(m2[:], pm2[:], channels=128,
                                           reduce_op=bass_isa.ReduceOp.max)
            negm2 = cp.tile([128, 1], F32)
            nc.vector.tensor_scalar_mul(negm2[:], m2[:], -1.0)
            e2 = cp.tile([128, VT], F32)
            es2 = cp.tile([128, 1], F32)
            nc.scalar.activation(e2[:], lg[:], AF.Exp, bias=negm2[:], scale=1.0,
                                 accum_out=es2[:])
            s2 = cp.tile([128, 1], F32)
            nc.gpsimd.partition_all_reduce(s2[:], es2[:], channels=128,
                                           reduce_op=bass_isa.ReduceOp.add)

            ag3_row = cp.tile([1, 2], F32)
            nc.scalar.copy(ag3_row[:, 0:1], m2[0:1, :])
            nc.scalar.copy(ag3_row[:, 1:2], s2[0:1, :])
            ag3_in = dp.tile([1, 2], F32)
            ag3_out = dp.tile([NC, 2], F32)
            nc.gpsimd.dma_start(ag3_in[:], ag3_row[:])
            nc.gpsimd.collective_compute(
                "AllGather", ALU.bypass, replica_groups=rg,
                ins=[ag3_in.opt()], outs=[ag3_out.opt()],
            )

            r3 = cp.tile([1, 2 * NC], F32)
            nc.sync.dma_start(r3[:], ag3_out[:].rearrange("a b -> (a b)")[None, :])
            m8 = r3[:].rearrange("a (n two) -> a n two", two=2)[:, :, 0]
            s8 = r3[:].rearrange("a (n two) -> a n two", two=2)[:, :, 1]
            gM2 = cp.tile([1, 1], F32)
            nc.vector.reduce_max(gM2[:], m8, axis=AX.X)
            d8 = cp.tile([1, NC], F32)
            nc.vector.tensor_scalar(out=d8[:], in0=m8, scalar1=gM2[:],
                                    scalar2=None, op0=ALU.subtract)
            nc.scalar.activation(d8[:], d8[:], AF.Exp)
            nc.vector.tensor_tensor(out=d8[:], in0=d8[:], in1=s8, op=ALU.mult)
            S2g = cp.tile([1, 1], F32)
            nc.vector.reduce_sum(S2g[:], d8[:], axis=AX.X)
            nc.scalar.activation(S2g[:], S2g[:], AF.Ln)
            logZ = cp.tile([1, 1], F32)
            nc.vector.tensor_add(out=logZ[:], in0=S2g[:], in1=gM2[:])
            logZ_col = cp.tile([128, 1], F32)
            nc.gpsimd.partition_broadcast(logZ_col[:], logZ[:])
            outt = cp.tile([128, VT], F32)
            nc.vector.tensor_scalar(out=outt[:], in0=lg[:], scalar1=logZ_col[:],
                                    scalar2=None, op0=ALU.subtract)
            nc.sync.dma_start(out_logits[:], outt[:])

    nc.finalize()
    return nc


def make_in_maps(inputs):
    tok = np.asarray(inputs["input_tok"]).astype(np.int32).reshape(1, 1)
    emb = np.ascontiguousarray(np.asarray(inputs["emb"], dtype=np.float32))
    lc = np.asarray(inputs["last_context"], dtype=np.float32).reshape(1, H)
    lh = np.asarray(inputs["last_hidden"], dtype=np.float32).reshape(1, H)
    encf = np.asarray(inputs["encoder_outputs"], dtype=np.float32).reshape(S, H)
    w_ih = np.asarray(inputs["w_ih"], dtype=np.float32)
    w_hh = np.asarray(inputs["w_hh"], dtype=np.float32)
    b_ih = np.asarray(inputs["b_ih"], dtype=np.float32)
    b_hh = np.asarray(inputs["b_hh"], dtype=np.float32)
    atten_W = np.asarray(inputs["atten_W"], dtype=np.float32)
    w_score = np.asarray(inputs["w_score"], dtype=np.float32)
    out_W = np.asarray(inputs["out_W"], dtype=np.float32)
    out_b = np.asarray(inputs["out_b"], dtype=np.float32)

    wih3 = w_ih.reshape(3, H, H2)
    whh3 = w_hh.reshape(3, H, H)
    bih3 = b_ih.reshape(3, H)
    bhh3 = b_hh.reshape(3, H)
    ws_t = np.ascontiguousarray(w_score.reshape(8, 128).T)

    W8 = np.zeros((NC * VS, H2), dtype=np.float32)
    W8[:V] = out_W
    b8 = np.full(NC * VS, NEG_BIG, dtype=np.float32)
    b8[:V] = out_b

    in_maps = []
    for c in range(NC):
        hs = slice(c * 128, (c + 1) * 128)
        in_maps.append({
            "tok": tok,
            "emb": emb,
            "lc": lc,
            "lh": lh,
            "lh_col": np.ascontiguousarray(lh[0, hs].reshape(128, 1)),
            "enc": np.ascontiguousarray(encf[c * SS : (c + 1) * SS]),
            "w_ihx": np.ascontiguousarray(wih3[:, hs, :].reshape(3 * 128, H2)),
            "w_hhx": np.ascontiguousarray(whh3[:, hs, :].reshape(3 * 128, H)),
            "b_ihx": np.ascontiguousarray(bih3[:, hs].T),
            "b_hhx": np.ascontiguousarray(bhh3[:, hs].T),
            "ws_t": ws_t,
            "awx": np.ascontiguousarray(atten_W[:, H + c * 128 : H + (c + 1) * 128]),
            "outWx": np.ascontiguousarray(W8[c * VS : (c + 1) * VS]),
            "outbx": np.ascontiguousarray(
                b8[c * VS : (c + 1) * VS].reshape(VT, 128).T
            ),
        })
    return in_maps


def assemble(results):
    logits = np.concatenate(
        [results[c]["out_logits"].T.reshape(VS) for c in range(NC)]
    )[:V].reshape(1, V)
    weights = np.concatenate(
        [results[c]["out_w"].T.reshape(SS) for c in range(NC)]
    ).reshape(1, 1, S)
    context = results[0]["out_context"].reshape(1, H)
    hidden = results[0]["out_hidden"].reshape(1, 1, H)
    return logits, context, hidden, weights


_NC_CACHE = None


def kernel(**inputs):
    global _NC_CACHE
    if _NC_CACHE is None:
        _NC_CACHE = build_nc()
    in_maps = make_in_maps(inputs)
    res = run_bass_kernel_spmd(_NC_CACHE, in_maps, core_ids=list(range(NC)),
                               trace=False)
    return assemble(res.results)


# revision 6
# speedup vs baseline: 1.2357x; 1.2357x over previous
"""DecoderRNN (single-step GRU + Bahdanau attention + vocab projection) on 8 TRN2 cores.

Sharding:
  - GRU: hidden dim row-sharded (each core owns a 128-slice of H for all 3 gates).
  - Attention score vector v2 = w_score^T @ atten_W[:, H:2H]: column-sharded
    (128 output cols per core). h_new and v2 are combined in one AllGather.
    Note softmax(scores + c) == softmax(scores) for any constant c, so the
    hidden-state contribution to the (tanh-free) attention scores cancels;
    only enc @ v2 matters.
  - encoder_outputs: sequence-sharded (512 rows per core); local softmax stats +
    partial context AllGathered and combined on every core.
  - out_W/out_b: vocab-sharded (6400 padded rows per core); log-softmax stats
    AllGathered for the global logsumexp.

Mat-vecs run as DVE tensor_mul + ScalarE activation(Copy, accum_out) pairs
(the fused TENSOR_TENSOR_REDUCE crashes this runtime), so the multiply and the
free-dim reduction pipeline on different engines under the out_W DMA stream.
"""

import numpy as np

import concourse.bass as bass
import concourse.mybir as mybir
import concourse.tile as tile
from concourse import bacc, bass_isa
from concourse.bass_utils import run_bass_kernel_spmd
from concourse.masks import make_identity

H = 1024
H2 = 2048
V = 50257
S = 4096
NC = 8
VS = 6400        # padded vocab rows per core (8*6400 = 51200 >= V)
VT = VS // 128   # 50 vocab tiles per core
SS = S // NC     # 512 encoder rows per core
ST = SS // 128   # 4 encoder tiles per core

F32 = mybir.dt.float32
I32 = mybir.dt.int32
AX = mybir.AxisListType
ALU = mybir.AluOpType
AF = mybir.ActivationFunctionType

NEG_BIG = -1.0e30


def build_nc(n_reps: int = 1, stream_bufs: int = 14):
    nc = bacc.Bacc(None, target_bir_lowering=False, num_devices=NC)

    # ---- per-core external inputs ----
    tok = nc.dram_tensor("tok", [1, 1], I32, kind="ExternalInput")
    emb = nc.dram_tensor("emb", [V, H], F32, kind="ExternalInput")
    lc = nc.dram_tensor("lc", [1, H], F32, kind="ExternalInput")
    lh = nc.dram_tensor("lh", [1, H], F32, kind="ExternalInput")
    lh_col = nc.dram_tensor("lh_col", [128, 1], F32, kind="ExternalInput")
    enc = nc.dram_tensor("enc", [SS, H], F32, kind="ExternalInput")
    w_ihx = nc.dram_tensor("w_ihx", [3 * 128, H2], F32, kind="ExternalInput")
    w_hhx = nc.dram_tensor("w_hhx", [3 * 128, H], F32, kind="ExternalInput")
    b_ihx = nc.dram_tensor("b_ihx", [128, 3], F32, kind="ExternalInput")
    b_hhx = nc.dram_tensor("b_hhx", [128, 3], F32, kind="ExternalInput")
    ws_t = nc.dram_tensor("ws_t", [128, 8], F32, kind="ExternalInput")
    awx = nc.dram_tensor("awx", [H, 128], F32, kind="ExternalInput")
    outWx = nc.dram_tensor("outWx", [VS, H2], F32, kind="ExternalInput")
    outbx = nc.dram_tensor("outbx", [128, VT], F32, kind="ExternalInput")

    # ---- per-core external outputs ----
    out_logits = nc.dram_tensor("out_logits", [128, VT], F32, kind="ExternalOutput")
    out_w = nc.dram_tensor("out_w", [128, ST], F32, kind="ExternalOutput")
    out_context = nc.dram_tensor("out_context", [1, H], F32, kind="ExternalOutput")
    out_hidden = nc.dram_tensor("out_hidden", [1, H], F32, kind="ExternalOutput")

    rg = [list(range(NC))]

    with tile.TileContext(nc) as tc:
        with (
            tc.tile_pool(name="small", bufs=1 if n_reps == 1 else 2) as cp,
            tc.tile_pool(name="stream", bufs=stream_bufs) as wp,
            tc.tile_pool(name="scratch", bufs=2) as sp,
            tc.tile_pool(name="psum", bufs=1 if n_reps == 1 else 2, space="PSUM") as pp,
            tc.tile_pool(name="dram", bufs=1, space="DRAM") as dp,
        ):
            for r in range(n_reps):
                _emit(nc, tc, cp, wp, sp, pp, dp, rg, r,
                      tok=tok, emb=emb, lc=lc, lh=lh, lh_col=lh_col, enc=enc,
                      w_ihx=w_ihx, w_hhx=w_hhx, b_ihx=b_ihx, b_hhx=b_hhx,
                      ws_t=ws_t, awx=awx, outWx=outWx, outbx=outbx,
                      out_logits=out_logits, out_w=out_w,
                      out_context=out_context, out_hidden=out_hidden)

    nc.finalize()
    return nc


def _emit(nc, tc, cp, wp, sp, pp, dp, rg, r, *, tok, emb, lc, lh, lh_col, enc,
          w_ihx, w_hhx, b_ihx, b_hhx, ws_t, awx, outWx, outbx,
          out_logits, out_w, out_context, out_hidden):
    cnt = [0]

    def ct(shape, dtype=F32, tag=None):
        cnt[0] += 1
        return cp.tile(shape, dtype, tag=tag, name=f"{tag}_r{r}_{cnt[0]}")

    def wt_tile():
        cnt[0] += 1
        return wp.tile([128, H2], F32, tag="wt", name=f"wt_r{r}_{cnt[0]}")

    # ---------- offsets for replicating gathers ----------
    tok_sb = ct([1, 1], I32, tag="tok_sb")
    nc.sync.dma_start(tok_sb[:], tok[:])
    tok_col = ct([128, 1], I32, tag="tok_col")
    nc.gpsimd.partition_broadcast(tok_col[:], tok_sb[:])
    zero_col = ct([128, 1], I32, tag="zero_col")
    nc.vector.memset(zero_col[:], 0)

    # ---------- broadcast-replicated input vectors ----------
    x_rep = ct([128, H2], tag="x_rep")
    nc.gpsimd.indirect_dma_start(
        out=x_rep[:, 0:H], out_offset=None, in_=emb[:],
        in_offset=bass.IndirectOffsetOnAxis(ap=tok_col[:], axis=0))
    nc.gpsimd.indirect_dma_start(
        out=x_rep[:, H:H2], out_offset=None, in_=lc[:],
        in_offset=bass.IndirectOffsetOnAxis(ap=zero_col[:], axis=0))
    h_rep = ct([128, H], tag="h_rep")
    nc.gpsimd.indirect_dma_start(
        out=h_rep[:], out_offset=None, in_=lh[:],
        in_offset=bass.IndirectOffsetOnAxis(ap=zero_col[:], axis=0))

    # ---------- GRU gate mat-vecs ----------
    bih_t = ct([128, 3], tag="bih_t")
    nc.sync.dma_start(bih_t[:], b_ihx[:])
    bhh_t = ct([128, 3], tag="bhh_t")
    nc.sync.dma_start(bhh_t[:], b_hhx[:])

    gi = ct([128, 3], tag="gi")
    gh = ct([128, 3], tag="gh")
    wih_ap = w_ihx[:].rearrange("(g p) k -> p g k", p=128)
    for g in range(3):
        wg = wt_tile()
        nc.sync.dma_start(wg[:], wih_ap[:, g])
        sc2 = sp.tile([128, H2], F32, tag="sc2", name=f"sc2_{r}_{cnt[0]}")
        nc.vector.tensor_mul(out=sc2[:], in0=wg[:], in1=x_rep[:])
        nc.scalar.activation(sc2[:], sc2[:], AF.Copy, accum_out=gi[:, g : g + 1])
    whh_ap = w_hhx[:].rearrange("(g p) k -> p g k", p=128)
    for g in range(3):
        wg = wt_tile()
        nc.sync.dma_start(wg[:, 0:H], whh_ap[:, g])
        sc1 = sp.tile([128, H], F32, tag="sc1", name=f"sc1_{r}_{cnt[0]}")
        nc.vector.tensor_mul(out=sc1[:], in0=wg[:, 0:H], in1=h_rep[:])
        nc.scalar.activation(sc1[:], sc1[:], AF.Copy, accum_out=gh[:, g : g + 1])
    nc.vector.tensor_add(out=gi[:], in0=gi[:], in1=bih_t[:])
    nc.vector.tensor_add(out=gh[:], in0=gh[:], in1=bhh_t[:])

    # ---------- gates ----------
    rz = ct([128, 2], tag="rz")
    nc.vector.tensor_add(out=rz[:], in0=gi[:, 0:2], in1=gh[:, 0:2])
    nc.scalar.activation(rz[:], rz[:], AF.Sigmoid)
    n_t = ct([128, 1], tag="n_t")
    nc.vector.tensor_mul(out=n_t[:], in0=rz[:, 0:1], in1=gh[:, 2:3])
    nc.vector.tensor_add(out=n_t[:], in0=n_t[:], in1=gi[:, 2:3])
    nc.scalar.activation(n_t[:], n_t[:], AF.Tanh)
    h_col = ct([128, 1], tag="h_col")
    nc.sync.dma_start(h_col[:], lh_col[:])
    hnew_col = ct([128, 1], tag="hnew_col")
    nc.vector.tensor_sub(out=hnew_col[:], in0=h_col[:], in1=n_t[:])
    nc.vector.tensor_mul(out=hnew_col[:], in0=hnew_col[:], in1=rz[:, 1:2])
    nc.vector.tensor_add(out=hnew_col[:], in0=hnew_col[:], in1=n_t[:])

    # ---------- v2 column shard ----------
    aw_t = wt_tile()
    aw_v = aw_t[:, 0:1024].rearrange("p (o n) -> p o n", o=8)
    nc.sync.dma_start(aw_v, awx[:].rearrange("(o p) n -> p o n", p=128))
    ws_sb = ct([128, 8], tag="ws_sb")
    nc.sync.dma_start(ws_sb[:], ws_t[:])
    ps_v = pp.tile([1, 128], F32, name=f"ps_v{r}")
    for o in range(8):
        nc.tensor.matmul(out=ps_v[:], lhsT=ws_sb[:, o : o + 1], rhs=aw_v[:, o],
                         start=(o == 0), stop=(o == 7))

    # ---------- h_new to row layout via PE transpose ----------
    ident = ct([128, 128], tag="ident")
    make_identity(nc, ident[:])
    ps_h = pp.tile([1, 128], F32, name=f"ps_h{r}")
    nc.tensor.matmul(out=ps_h[:], lhsT=hnew_col[:], rhs=ident[:], start=True, stop=True)

    # ---------- AllGather [h_new_slice | v2_slice] ----------
    ag1_row = ct([1, 256], tag="ag1_row")
    nc.scalar.copy(ag1_row[:, 0:128], ps_h[:])
    nc.scalar.copy(ag1_row[:, 128:256], ps_v[:])
    ag1_in = dp.tile([1, 256], F32, name=f"ag1_in{r}")
    ag1_out = dp.tile([NC, 256], F32, name=f"ag1_out{r}")
    nc.gpsimd.dma_start(ag1_in[:], ag1_row[:])
    nc.gpsimd.collective_compute(
        "AllGather", ALU.bypass, replica_groups=rg,
        ins=[ag1_in.opt()], outs=[ag1_out.opt()])

    # extract h_new row + v2 row (strided DRAM->SBUF), replicate on partitions
    hfull_row = ct([1, H], tag="hfull_row")
    nc.sync.dma_start(hfull_row[:], ag1_out[:, 0:128])
    v2_row = ct([1, H], tag="v2_row")
    nc.sync.dma_start(v2_row[:], ag1_out[:, 128:256])
    nc.sync.dma_start(out_hidden[:], hfull_row[:])

    xout_rep = ct([128, H2], tag="xout_rep")
    nc.gpsimd.partition_broadcast(xout_rep[:, 0:H], hfull_row[:])
    v2_rep = ct([128, H], tag="v2_rep")
    nc.gpsimd.partition_broadcast(v2_rep[:], v2_row[:])

    # ---------- attention scores + local softmax + partial context ----------
    enc_ap = enc[:].rearrange("(t p) k -> p t k", p=128)
    enc_tiles = []
    for t in range(ST):
        et = wt_tile()
        nc.sync.dma_start(et[:, 0:H], enc_ap[:, t])
        enc_tiles.append(et)
    sc = ct([128, ST], tag="sc")
    for t in range(ST):
        sc1 = sp.tile([128, H], F32, tag="sc1", name=f"sc1_{r}_{cnt[0]}")
        nc.vector.tensor_mul(out=sc1[:], in0=enc_tiles[t][:, 0:H], in1=v2_rep[:])
        nc.scalar.activation(sc1[:], sc1[:], AF.Copy, accum_out=sc[:, t : t + 1])
    pmax = ct([128, 1], tag="pmax")
    nc.vector.reduce_max(pmax[:], sc[:], axis=AX.X)
    m_loc = ct([128, 1], tag="m_loc")
    nc.gpsimd.partition_all_reduce(m_loc[:], pmax[:], channels=128,
                                   reduce_op=bass_isa.ReduceOp.max)
    negm = ct([128, 1], tag="negm")
    nc.vector.tensor_scalar_mul(negm[:], m_loc[:], -1.0)
    e_t = ct([128, ST], tag="e_t")
    psum_part = ct([128, 1], tag="psum_part")
    nc.scalar.activation(e_t[:], sc[:], AF.Exp, bias=negm[:], scale=1.0,
                         accum_out=psum_part[:])
    s_loc = ct([128, 1], tag="s_loc")
    nc.gpsimd.partition_all_reduce(s_loc[:], psum_part[:], channels=128,
                                   reduce_op=bass_isa.ReduceOp.add)
    ps_pa = pp.tile([1, 512], F32, name=f"ps_pa{r}")
    ps_pb = pp.tile([1, 512], F32, name=f"ps_pb{r}")
    for t in range(ST):
        nc.tensor.matmul(out=ps_pa[:], lhsT=e_t[:, t : t + 1],
                         rhs=enc_tiles[t][:, 0:512], start=(t == 0), stop=(t == ST - 1))
    for t in range(ST):
        nc.tensor.matmul(out=ps_pb[:], lhsT=e_t[:, t : t + 1],
                         rhs=enc_tiles[t][:, 512:H], start=(t == 0), stop=(t == ST - 1))

    # ---------- AllGather [m_loc, s_loc, pctx] ----------
    ag2_row = ct([1, 2 + H], tag="ag2_row")
    nc.scalar.copy(ag2_row[:, 0:1], m_loc[0:1, :])
    nc.scalar.copy(ag2_row[:, 1:2], s_loc[0:1, :])
    nc.scalar.copy(ag2_row[:, 2 : 2 + 512], ps_pa[:])
    nc.scalar.copy(ag2_row[:, 2 + 512 : 2 + H], ps_pb[:])
    ag2_in = dp.tile([1, 2 + H], F32, name=f"ag2_in{r}")
    ag2_out = dp.tile([NC, 2 + H], F32, name=f"ag2_out{r}")
    nc.gpsimd.dma_start(ag2_in[:], ag2_row[:])
    nc.gpsimd.collective_compute(
        "AllGather", ALU.bypass, replica_groups=rg,
        ins=[ag2_in.opt()], outs=[ag2_out.opt()])

    # ---------- combine: global softmax + context ----------
    mrow = ct([1, NC], tag="mrow")
    nc.sync.dma_start(mrow[:], ag2_out[:, 0:1])
    srow = ct([1, NC], tag="srow")
    nc.sync.dma_start(srow[:], ag2_out[:, 1:2])
    pctx_t = ct([NC, H], tag="pctx_t")
    nc.sync.dma_start(pctx_t[:], ag2_out[:, 2 : 2 + H])

    gM = ct([1, 1], tag="gM")
    nc.vector.reduce_max(gM[:], mrow[:], axis=AX.X)
    dm = ct([1, NC], tag="dm")
    nc.vector.tensor_scalar(out=dm[:], in0=mrow[:], scalar1=gM[:],
                            scalar2=None, op0=ALU.subtract)
    al = ct([1, NC], tag="al")
    nc.scalar.activation(al[:], dm[:], AF.Exp)
    wsum = ct([1, NC], tag="wsum")
    nc.vector.tensor_mul(out=wsum[:], in0=al[:], in1=srow[:])
    S_tot = ct([1, 1], tag="S_tot")
    nc.vector.reduce_sum(S_tot[:], wsum[:], axis=AX.X)
    rS = ct([1, 1], tag="rS")
    nc.vector.reciprocal(rS[:], S_tot[:])
    grow = ct([1, NC], tag="grow")
    nc.vector.tensor_scalar_mul(grow[:], al[:], rS[:])

    ones11 = ct([1, 1], tag="ones11")
    nc.vector.memset(ones11[:], 1.0)
    ps_g = pp.tile([NC, 1], F32, name=f"ps_g{r}")
    nc.tensor.matmul(out=ps_g[:], lhsT=grow[:], rhs=ones11[:], start=True, stop=True)
    gcol = ct([NC, 1], tag="gcol")
    nc.scalar.copy(gcol[:], ps_g[:])

    ps_ca = pp.tile([1, 512], F32, name=f"ps_ca{r}")
    ps_cb = pp.tile([1, 512], F32, name=f"ps_cb{r}")
    nc.tensor.matmul(out=ps_ca[:], lhsT=gcol[:], rhs=pctx_t[:, 0:512], start=True, stop=True)
    nc.tensor.matmul(out=ps_cb[:], lhsT=gcol[:], rhs=pctx_t[:, 512:H], start=True, stop=True)
    ctx_row = ct([1, H], tag="ctx_row")
    nc.scalar.copy(ctx_row[:, 0:512], ps_ca[:])
    nc.scalar.copy(ctx_row[:, 512:H], ps_cb[:])
    nc.sync.dma_start(out_context[:], ctx_row[:])

    # attention weights output
    dmine = ct([1, 1], tag="dmine")
    nc.vector.tensor_sub(out=dmine[:], in0=m_loc[0:1, :], in1=gM[:])
    nc.scalar.activation(dmine[:], dmine[:], AF.Exp)
    nc.vector.tensor_mul(out=dmine[:], in0=dmine[:], in1=rS[:])
    gm_col = ct([128, 1], tag="gm_col")
    nc.gpsimd.partition_broadcast(gm_col[:], dmine[:])
    w_tile = ct([128, ST], tag="w_tile")
    nc.vector.tensor_scalar_mul(w_tile[:], e_t[:], gm_col[:])
    nc.sync.dma_start(out_w[:], w_tile[:])

    # replicate context into xout_rep
    nc.gpsimd.partition_broadcast(xout_rep[:, H:H2], ctx_row[:])

    # ---------- vocab shard logits ----------
    outb_t = ct([128, VT], tag="outb_t")
    nc.sync.dma_start(outb_t[:], outbx[:])
    lg = ct([128, VT], tag="lg")
    for i in range(VT):
        wtile = wt_tile()
        nc.sync.dma_start(wtile[:], outWx[128 * i : 128 * (i + 1), :])
        sc2 = sp.tile([128, H2], F32, tag="sc2", name=f"sc2_{r}_{cnt[0]}")
        nc.vector.tensor_mul(out=sc2[:], in0=wtile[:], in1=xout_rep[:])
        nc.scalar.activation(sc2[:], sc2[:], AF.Copy, accum_out=lg[:, i : i + 1])
    nc.vector.tensor_add(out=lg[:], in0=lg[:], in1=outb_t[:])

    # ---------- log-softmax ----------
    pm2 = ct([128, 1], tag="pm2")
    nc.vector.reduce_max(pm2[:], lg[:], axis=AX.X)
    m2 = ct([128, 1], tag="m2")
    nc.gpsimd.partition_all_reduce(m2[:], pm2[:], channels=128,
                                   reduce_op=bass_isa.ReduceOp.max)
    negm2 = ct([128, 1], tag="negm2")
    nc.vector.tensor_scalar_mul(negm2[:], m2[:], -1.0)
    e2 = ct([128, VT], tag="e2")
    es2 = ct([128, 1], tag="es2")
    nc.scalar.activation(e2[:], lg[:], AF.Exp, bias=negm2[:], scale=1.0,
                         accum_out=es2[:])
    s2 = ct([128, 1], tag="s2")
    nc.gpsimd.partition_all_reduce(s2[:], es2[:], channels=128,
                                   reduce_op=bass_isa.ReduceOp.add)

    ag3_row = ct([1, 2], tag="ag3_row")
    nc.scalar.copy(ag3_row[:, 0:1], m2[0:1, :])
    nc.scalar.copy(ag3_row[:, 1:2], s2[0:1, :])
    ag3_in = dp.tile([1, 2], F32, name=f"ag3_in{r}")
    ag3_out = dp.tile([NC, 2], F32, name=f"ag3_out{r}")
    nc.gpsimd.dma_start(ag3_in[:], ag3_row[:])
    nc.gpsimd.collective_compute(
        "AllGather", ALU.bypass, replica_groups=rg,
        ins=[ag3_in.opt()], outs=[ag3_out.opt()])

    r3 = ct([1, 2 * NC], tag="r3")
    nc.sync.dma_start(r3[:], ag3_out[:].rearrange("a b -> (a b)")[None, :])
    m8 = r3[:].rearrange("a (n two) -> a n two", two=2)[:, :, 0]
    s8 = r3[:].rearrange("a (n two) -> a n two", two=2)[:, :, 1]
    gM2 = ct([1, 1], tag="gM2")
    nc.vector.reduce_max(gM2[:], m8, axis=AX.X)
    d8 = ct([1, NC], tag="d8")
    nc.vector.tensor_scalar(out=d8[:], in0=m8, scalar1=gM2[:],
                            scalar2=None, op0=ALU.subtract)
    nc.scalar.activation(d8[:], d8[:], AF.Exp)
    nc.vector.tensor_tensor(out=d8[:], in0=d8[:], in1=s8, op=ALU.mult)
    S2g = ct([1, 1], tag="S2g")
    nc.vector.reduce_sum(S2g[:], d8[:], axis=AX.X)
    nc.scalar.activation(S2g[:], S2g[:], AF.Ln)
    logZ = ct([1, 1], tag="logZ")
    nc.vector.tensor_add(out=logZ[:], in0=S2g[:], in1=gM2[:])
    logZ_col = ct([128, 1], tag="logZ_col")
    nc.gpsimd.partition_broadcast(logZ_col[:], logZ[:])
    outt = ct([128, VT], tag="outt")
    nc.vector.tensor_scalar(out=outt[:], in0=lg[:], scalar1=logZ_col[:],
                            scalar2=None, op0=ALU.subtract)
    nc.sync.dma_start(out_logits[:], outt[:])


def make_in_maps(inputs):
    tok = np.asarray(inputs["input_tok"]).astype(np.int32).reshape(1, 1)
    emb = np.ascontiguousarray(np.asarray(inputs["emb"], dtype=np.float32))
    lc = np.asarray(inputs["last_context"], dtype=np.float32).reshape(1, H)
    lh = np.asarray(inputs["last_hidden"], dtype=np.float32).reshape(1, H)
    encf = np.asarray(inputs["encoder_outputs"], dtype=np.float32).reshape(S, H)
    w_ih = np.asarray(inputs["w_ih"], dtype=np.float32)
    w_hh = np.asarray(inputs["w_hh"], dtype=np.float32)
    b_ih = np.asarray(inputs["b_ih"], dtype=np.float32)
    b_hh = np.asarray(inputs["b_hh"], dtype=np.float32)
    atten_W = np.asarray(inputs["atten_W"], dtype=np.float32)
    w_score = np.asarray(inputs["w_score"], dtype=np.float32)
    out_W = np.asarray(inputs["out_W"], dtype=np.float32)
    out_b = np.asarray(inputs["out_b"], dtype=np.float32)

    wih3 = w_ih.reshape(3, H, H2)
    whh3 = w_hh.reshape(3, H, H)
    bih3 = b_ih.reshape(3, H)
    bhh3 = b_hh.reshape(3, H)
    ws_t = np.ascontiguousarray(w_score.reshape(8, 128).T)

    W8 = np.zeros((NC * VS, H2), dtype=np.float32)
    W8[:V] = out_W
    b8 = np.full(NC * VS, NEG_BIG, dtype=np.float32)
    b8[:V] = out_b

    in_maps = []
    for c in range(NC):
        hs = slice(c * 128, (c + 1) * 128)
        in_maps.append({
            "tok": tok,
            "emb": emb,
            "lc": lc,
            "lh": lh,
            "lh_col": np.ascontiguousarray(lh[0, hs].reshape(128, 1)),
            "enc": np.ascontiguousarray(encf[c * SS : (c + 1) * SS]),
            "w_ihx": np.ascontiguousarray(wih3[:, hs, :].reshape(3 * 128, H2)),
            "w_hhx": np.ascontiguousarray(whh3[:, hs, :].reshape(3 * 128, H)),
            "b_ihx": np.ascontiguousarray(bih3[:, hs].T),
            "b_hhx": np.ascontiguousarray(bhh3[:, hs].T),
            "ws_t": ws_t,
            "awx": np.ascontiguousarray(atten_W[:, H + c * 128 : H + (c + 1) * 128]),
            "outWx": np.ascontiguousarray(W8[c * VS : (c + 1) * VS]),
            "outbx": np.ascontiguousarray(
                b8[c * VS : (c + 1) * VS].reshape(VT, 128).T
            ),
        })
    return in_maps


def assemble(results):
    logits = np.concatenate(
        [results[c]["out_logits"].T.reshape(VS) for c in range(NC)]
    )[:V].reshape(1, V)
    weights = np.concatenate(
        [results[c]["out_w"].T.reshape(SS) for c in range(NC)]
    ).reshape(1, 1, S)
    context = results[0]["out_context"].reshape(1, H)
    hidden = results[0]["out_hidden"].reshape(1, 1, H)
    return logits, context, hidden, weights


_NC_CACHE = None


def kernel(**inputs):
    global _NC_CACHE
    if _NC_CACHE is None:
        _NC_CACHE = build_nc()
    in_maps = make_in_maps(inputs)
    res = run_bass_kernel_spmd(_NC_CACHE, in_maps, core_ids=list(range(NC)),
                               trace=False)
    return assemble(res.results)


# revision 8
# speedup vs baseline: 504.8278x; 408.5411x over previous
"""DecoderRNN (single-step GRU + Bahdanau attention + vocab projection) on 8 TRN2 cores.

Sharding:
  - GRU: hidden dim row-sharded (each core owns a 128-slice of H for all 3 gates).
  - Attention score vector v2 = w_score^T @ atten_W[:, H:2H]: column-sharded
    (128 output cols per core). h_new and v2 are combined in one AllGather.
    Note softmax(scores + c) == softmax(scores) for any constant c, so the
    hidden-state contribution to the (tanh-free) attention scores cancels;
    only enc @ v2 matters.
  - encoder_outputs: sequence-sharded (512 rows per core); local softmax stats +
    partial context AllGathered and combined on every core.
  - out_W/out_b: vocab-sharded (6400 padded rows per core); log-softmax stats
    AllGathered for the global logsumexp.

Mat-vecs run as DVE tensor_mul + ScalarE activation(Copy, accum_out) pairs
(the fused TENSOR_TENSOR_REDUCE crashes this runtime), so the multiply and the
free-dim reduction pipeline on different engines under the out_W DMA stream.
"""

import numpy as np

import concourse.bass as bass
import concourse.mybir as mybir
import concourse.tile as tile
from concourse import bacc, bass_isa
from concourse.bass_utils import run_bass_kernel_spmd
from concourse.masks import make_identity

H = 1024
H2 = 2048
V = 50257
S = 4096
NC = 8
VS = 6400        # padded vocab rows per core (8*6400 = 51200 >= V)
VT = VS // 128   # 50 vocab tiles per core
SS = S // NC     # 512 encoder rows per core
ST = SS // 128   # 4 encoder tiles per core

F32 = mybir.dt.float32
I32 = mybir.dt.int32
AX = mybir.AxisListType
ALU = mybir.AluOpType
AF = mybir.ActivationFunctionType

NEG_BIG = -1.0e30


def build_nc(n_reps: int = 1, stream_bufs: int = 14):
    nc = bacc.Bacc(None, target_bir_lowering=False, num_devices=NC)

    # ---- per-core external inputs ----
    tok = nc.dram_tensor("tok", [1, 1], I32, kind="ExternalInput")
    emb = nc.dram_tensor("emb", [V, H], F32, kind="ExternalInput")
    lc = nc.dram_tensor("lc", [1, H], F32, kind="ExternalInput")
    lh = nc.dram_tensor("lh", [1, H], F32, kind="ExternalInput")
    lh_col = nc.dram_tensor("lh_col", [128, 1], F32, kind="ExternalInput")
    enc = nc.dram_tensor("enc", [SS, H], F32, kind="ExternalInput")
    w_ihx = nc.dram_tensor("w_ihx", [3 * 128, H2], F32, kind="ExternalInput")
    w_hhx = nc.dram_tensor("w_hhx", [3 * 128, H], F32, kind="ExternalInput")
    b_ihx = nc.dram_tensor("b_ihx", [128, 3], F32, kind="ExternalInput")
    b_hhx = nc.dram_tensor("b_hhx", [128, 3], F32, kind="ExternalInput")
    ws_t = nc.dram_tensor("ws_t", [128, 8], F32, kind="ExternalInput")
    awx = nc.dram_tensor("awx", [H, 128], F32, kind="ExternalInput")
    outWx = nc.dram_tensor("outWx", [VS, H2], F32, kind="ExternalInput")
    outbx = nc.dram_tensor("outbx", [128, VT], F32, kind="ExternalInput")

    # ---- per-core external outputs ----
    out_logits = nc.dram_tensor("out_logits", [128, VT], F32, kind="ExternalOutput")
    out_w = nc.dram_tensor("out_w", [128, ST], F32, kind="ExternalOutput")
    out_context = nc.dram_tensor("out_context", [1, H], F32, kind="ExternalOutput")
    out_hidden = nc.dram_tensor("out_hidden", [1, H], F32, kind="ExternalOutput")

    rg = [list(range(NC))]

    with tile.TileContext(nc) as tc:
        with (
            tc.tile_pool(name="small", bufs=1 if n_reps == 1 else 2) as cp,
            tc.tile_pool(name="stream", bufs=stream_bufs) as wp,
            tc.tile_pool(name="scratch", bufs=2) as sp,
            tc.tile_pool(name="psum", bufs=1, space="PSUM") as pp,
            tc.tile_pool(name="dram", bufs=1, space="DRAM") as dp,
        ):
            for r in range(n_reps):
                _emit(nc, tc, cp, wp, sp, pp, dp, rg, r,
                      tok=tok, emb=emb, lc=lc, lh=lh, lh_col=lh_col, enc=enc,
                      w_ihx=w_ihx, w_hhx=w_hhx, b_ihx=b_ihx, b_hhx=b_hhx,
                      ws_t=ws_t, awx=awx, outWx=outWx, outbx=outbx,
                      out_logits=out_logits, out_w=out_w,
                      out_context=out_context, out_hidden=out_hidden)

    nc.finalize()
    return nc


def _emit(nc, tc, cp, wp, sp, pp, dp, rg, r, *, tok, emb, lc, lh, lh_col, enc,
          w_ihx, w_hhx, b_ihx, b_hhx, ws_t, awx, outWx, outbx,
          out_logits, out_w, out_context, out_hidden):
    cnt = [0]

    def ct(shape, dtype=F32, tag=None):
        cnt[0] += 1
        return cp.tile(shape, dtype, tag=tag, name=f"{tag}_r{r}_{cnt[0]}")

    def wt_tile():
        cnt[0] += 1
        return wp.tile([128, H2], F32, tag="wt", name=f"wt_r{r}_{cnt[0]}")

    # ---------- offsets for replicating gathers ----------
    tok_sb = ct([1, 1], I32, tag="tok_sb")
    nc.sync.dma_start(tok_sb[:], tok[:])
    tok_col = ct([128, 1], I32, tag="tok_col")
    nc.gpsimd.partition_broadcast(tok_col[:], tok_sb[:])
    zero_col = ct([128, 1], I32, tag="zero_col")
    nc.vector.memset(zero_col[:], 0)

    # ---------- broadcast-replicated input vectors ----------
    x_rep = ct([128, H2], tag="x_rep")
    nc.gpsimd.indirect_dma_start(
        out=x_rep[:, 0:H], out_offset=None, in_=emb[:],
        in_offset=bass.IndirectOffsetOnAxis(ap=tok_col[:], axis=0))
    nc.gpsimd.indirect_dma_start(
        out=x_rep[:, H:H2], out_offset=None, in_=lc[:],
        in_offset=bass.IndirectOffsetOnAxis(ap=zero_col[:], axis=0))
    h_rep = ct([128, H], tag="h_rep")
    nc.gpsimd.indirect_dma_start(
        out=h_rep[:], out_offset=None, in_=lh[:],
        in_offset=bass.IndirectOffsetOnAxis(ap=zero_col[:], axis=0))

    # ---------- GRU gate mat-vecs ----------
    bih_t = ct([128, 3], tag="bih_t")
    nc.sync.dma_start(bih_t[:], b_ihx[:])
    bhh_t = ct([128, 3], tag="bhh_t")
    nc.sync.dma_start(bhh_t[:], b_hhx[:])

    gi = ct([128, 3], tag="gi")
    gh = ct([128, 3], tag="gh")
    wih_ap = w_ihx[:].rearrange("(g p) k -> p g k", p=128)
    for g in range(3):
        wg = wt_tile()
        nc.sync.dma_start(wg[:], wih_ap[:, g])
        sc2 = sp.tile([128, H2], F32, tag="sc2", name=f"sc2_{r}_{cnt[0]}")
        nc.vector.tensor_mul(out=sc2[:], in0=wg[:], in1=x_rep[:])
        nc.scalar.activation(sc2[:], sc2[:], AF.Copy, accum_out=gi[:, g : g + 1])
    whh_ap = w_hhx[:].rearrange("(g p) k -> p g k", p=128)
    for g in range(3):
        wg = wt_tile()
        nc.sync.dma_start(wg[:, 0:H], whh_ap[:, g])
        sc1 = sp.tile([128, H], F32, tag="sc1", name=f"sc1_{r}_{cnt[0]}")
        nc.vector.tensor_mul(out=sc1[:], in0=wg[:, 0:H], in1=h_rep[:])
        nc.scalar.activation(sc1[:], sc1[:], AF.Copy, accum_out=gh[:, g : g + 1])
    nc.vector.tensor_add(out=gi[:], in0=gi[:], in1=bih_t[:])
    nc.vector.tensor_add(out=gh[:], in0=gh[:], in1=bhh_t[:])

    # ---------- gates ----------
    rz = ct([128, 2], tag="rz")
    nc.vector.tensor_add(out=rz[:], in0=gi[:, 0:2], in1=gh[:, 0:2])
    nc.scalar.activation(rz[:], rz[:], AF.Sigmoid)
    n_t = ct([128, 1], tag="n_t")
    nc.vector.tensor_mul(out=n_t[:], in0=rz[:, 0:1], in1=gh[:, 2:3])
    nc.vector.tensor_add(out=n_t[:], in0=n_t[:], in1=gi[:, 2:3])
    nc.scalar.activation(n_t[:], n_t[:], AF.Tanh)
    h_col = ct([128, 1], tag="h_col")
    nc.sync.dma_start(h_col[:], lh_col[:])
    hnew_col = ct([128, 1], tag="hnew_col")
    nc.vector.tensor_sub(out=hnew_col[:], in0=h_col[:], in1=n_t[:])
    nc.vector.tensor_mul(out=hnew_col[:], in0=hnew_col[:], in1=rz[:, 1:2])
    nc.vector.tensor_add(out=hnew_col[:], in0=hnew_col[:], in1=n_t[:])

    # ---------- v2 column shard ----------
    aw_t = wt_tile()
    aw_v = aw_t[:, 0:1024].rearrange("p (o n) -> p o n", o=8)
    nc.sync.dma_start(aw_v, awx[:].rearrange("(o p) n -> p o n", p=128))
    ws_sb = ct([128, 8], tag="ws_sb")
    nc.sync.dma_start(ws_sb[:], ws_t[:])
    ps_v = pp.tile([1, 128], F32, name=f"ps_v{r}", tag="ps_v")
    for o in range(8):
        nc.tensor.matmul(out=ps_v[:], lhsT=ws_sb[:, o : o + 1], rhs=aw_v[:, o],
                         start=(o == 0), stop=(o == 7))

    # ---------- h_new to row layout via PE transpose ----------
    ident = ct([128, 128], tag="ident")
    make_identity(nc, ident[:])
    ps_h = pp.tile([1, 128], F32, name=f"ps_h{r}", tag="ps_h")
    nc.tensor.matmul(out=ps_h[:], lhsT=hnew_col[:], rhs=ident[:], start=True, stop=True)

    # ---------- AllGather [h_new_slice | v2_slice] ----------
    ag1_row = ct([1, 256], tag="ag1_row")
    nc.scalar.copy(ag1_row[:, 0:128], ps_h[:])
    nc.scalar.copy(ag1_row[:, 128:256], ps_v[:])
    ag1_in = dp.tile([1, 256], F32, name=f"ag1_in{r}")
    ag1_out = dp.tile([NC, 256], F32, name=f"ag1_out{r}")
    nc.gpsimd.dma_start(ag1_in[:], ag1_row[:])
    nc.gpsimd.collective_compute(
        "AllGather", ALU.bypass, replica_groups=rg,
        ins=[ag1_in.opt()], outs=[ag1_out.opt()])

    # extract h_new row + v2 row (strided DRAM->SBUF), replicate on partitions
    hfull_row = ct([1, H], tag="hfull_row")
    nc.sync.dma_start(hfull_row[:], ag1_out[:, 0:128])
    v2_row = ct([1, H], tag="v2_row")
    nc.sync.dma_start(v2_row[:], ag1_out[:, 128:256])
    nc.sync.dma_start(out_hidden[:], hfull_row[:])

    xout_rep = ct([128, H2], tag="xout_rep")
    nc.gpsimd.partition_broadcast(xout_rep[:, 0:H], hfull_row[:])
    v2_rep = ct([128, H], tag="v2_rep")
    nc.gpsimd.partition_broadcast(v2_rep[:], v2_row[:])

    # ---------- attention scores + local softmax + partial context ----------
    enc_ap = enc[:].rearrange("(t p) k -> p t k", p=128)
    enc_tiles = []
    for t in range(ST):
        et = wt_tile()
        nc.sync.dma_start(et[:, 0:H], enc_ap[:, t])
        enc_tiles.append(et)
    sc = ct([128, ST], tag="sc")
    for t in range(ST):
        sc1 = sp.tile([128, H], F32, tag="sc1", name=f"sc1_{r}_{cnt[0]}")
        nc.vector.tensor_mul(out=sc1[:], in0=enc_tiles[t][:, 0:H], in1=v2_rep[:])
        nc.scalar.activation(sc1[:], sc1[:], AF.Copy, accum_out=sc[:, t : t + 1])
    pmax = ct([128, 1], tag="pmax")
    nc.vector.reduce_max(pmax[:], sc[:], axis=AX.X)
    m_loc = ct([128, 1], tag="m_loc")
    nc.gpsimd.partition_all_reduce(m_loc[:], pmax[:], channels=128,
                                   reduce_op=bass_isa.ReduceOp.max)
    negm = ct([128, 1], tag="negm")
    nc.vector.tensor_scalar_mul(negm[:], m_loc[:], -1.0)
    e_t = ct([128, ST], tag="e_t")
    psum_part = ct([128, 1], tag="psum_part")
    nc.scalar.activation(e_t[:], sc[:], AF.Exp, bias=negm[:], scale=1.0,
                         accum_out=psum_part[:])
    s_loc = ct([128, 1], tag="s_loc")
    nc.gpsimd.partition_all_reduce(s_loc[:], psum_part[:], channels=128,
                                   reduce_op=bass_isa.ReduceOp.add)
    ps_pa = pp.tile([1, 512], F32, name=f"ps_pa{r}", tag="ps_pa")
    ps_pb = pp.tile([1, 512], F32, name=f"ps_pb{r}", tag="ps_pb")
    for t in range(ST):
        nc.tensor.matmul(out=ps_pa[:], lhsT=e_t[:, t : t + 1],
                         rhs=enc_tiles[t][:, 0:512], start=(t == 0), stop=(t == ST - 1))
    for t in range(ST):
        nc.tensor.matmul(out=ps_pb[:], lhsT=e_t[:, t : t + 1],
                         rhs=enc_tiles[t][:, 512:H], start=(t == 0), stop=(t == ST - 1))

    # ---------- AllGather [m_loc, s_loc, pctx] ----------
    ag2_row = ct([1, 2 + H], tag="ag2_row")
    nc.scalar.copy(ag2_row[:, 0:1], m_loc[0:1, :])
    nc.scalar.copy(ag2_row[:, 1:2], s_loc[0:1, :])
    nc.scalar.copy(ag2_row[:, 2 : 2 + 512], ps_pa[:])
    nc.scalar.copy(ag2_row[:, 2 + 512 : 2 + H], ps_pb[:])
    ag2_in = dp.tile([1, 2 + H], F32, name=f"ag2_in{r}")
    ag2_out = dp.tile([NC, 2 + H], F32, name=f"ag2_out{r}")
    nc.gpsimd.dma_start(ag2_in[:], ag2_row[:])
    nc.gpsimd.collective_compute(
        "AllGather", ALU.bypass, replica_groups=rg,
        ins=[ag2_in.opt()], outs=[ag2_out.opt()])

    # ---------- combine: global softmax + context ----------
    mrow = ct([1, NC], tag="mrow")
    nc.sync.dma_start(mrow[:], ag2_out[:, 0:1])
    srow = ct([1, NC], tag="srow")
    nc.sync.dma_start(srow[:], ag2_out[:, 1:2])
    pctx_t = ct([NC, H], tag="pctx_t")
    nc.sync.dma_start(pctx_t[:], ag2_out[:, 2 : 2 + H])

    gM = ct([1, 1], tag="gM")
    nc.vector.reduce_max(gM[:], mrow[:], axis=AX.X)
    dm = ct([1, NC], tag="dm")
    nc.vector.tensor_scalar(out=dm[:], in0=mrow[:], scalar1=gM[:],
                            scalar2=None, op0=ALU.subtract)
    al = ct([1, NC], tag="al")
    nc.scalar.activation(al[:], dm[:], AF.Exp)
    wsum = ct([1, NC], tag="wsum")
    nc.vector.tensor_mul(out=wsum[:], in0=al[:], in1=srow[:])
    S_tot = ct([1, 1], tag="S_tot")
    nc.vector.reduce_sum(S_tot[:], wsum[:], axis=AX.X)
    rS = ct([1, 1], tag="rS")
    nc.vector.reciprocal(rS[:], S_tot[:])
    grow = ct([1, NC], tag="grow")
    nc.vector.tensor_scalar_mul(grow[:], al[:], rS[:])

    ones11 = ct([1, 1], tag="ones11")
    nc.vector.memset(ones11[:], 1.0)
    ps_g = pp.tile([NC, 1], F32, name=f"ps_g{r}", tag="ps_g")
    nc.tensor.matmul(out=ps_g[:], lhsT=grow[:], rhs=ones11[:], start=True, stop=True)
    gcol = ct([NC, 1], tag="gcol")
    nc.scalar.copy(gcol[:], ps_g[:])

    ps_ca = pp.tile([1, 512], F32, name=f"ps_ca{r}", tag="ps_ca")
    ps_cb = pp.tile([1, 512], F32, name=f"ps_cb{r}", tag="ps_cb")
    nc.tensor.matmul(out=ps_ca[:], lhsT=gcol[:], rhs=pctx_t[:, 0:512], start=True, stop=True)
    nc.tensor.matmul(out=ps_cb[:], lhsT=gcol[:], rhs=pctx_t[:, 512:H], start=True, stop=True)
    ctx_row = ct([1, H], tag="ctx_row")
    nc.scalar.copy(ctx_row[:, 0:512], ps_ca[:])
    nc.scalar.copy(ctx_row[:, 512:H], ps_cb[:])
    nc.sync.dma_start(out_context[:], ctx_row[:])

    # attention weights output
    dmine = ct([1, 1], tag="dmine")
    nc.vector.tensor_sub(out=dmine[:], in0=m_loc[0:1, :], in1=gM[:])
    nc.scalar.activation(dmine[:], dmine[:], AF.Exp)
    nc.vector.tensor_mul(out=dmine[:], in0=dmine[:], in1=rS[:])
    gm_col = ct([128, 1], tag="gm_col")
    nc.gpsimd.partition_broadcast(gm_col[:], dmine[:])
    w_tile = ct([128, ST], tag="w_tile")
    nc.vector.tensor_scalar_mul(w_tile[:], e_t[:], gm_col[:])
    nc.sync.dma_start(out_w[:], w_tile[:])

    # replicate context into xout_rep
    nc.gpsimd.partition_broadcast(xout_rep[:, H:H2], ctx_row[:])

    # ---------- vocab shard logits ----------
    outb_t = ct([128, VT], tag="outb_t")
    nc.sync.dma_start(outb_t[:], outbx[:])
    lg = ct([128, VT], tag="lg")
    for i in range(VT):
        wtile = wt_tile()
        nc.sync.dma_start(wtile[:], outWx[128 * i : 128 * (i + 1), :])
        sc2 = sp.tile([128, H2], F32, tag="sc2", name=f"sc2_{r}_{cnt[0]}")
        nc.vector.tensor_mul(out=sc2[:], in0=wtile[:], in1=xout_rep[:])
        nc.scalar.activation(sc2[:], sc2[:], AF.Copy, accum_out=lg[:, i : i + 1])
    nc.vector.tensor_add(out=lg[:], in0=lg[:], in1=outb_t[:])

    # ---------- log-softmax ----------
    pm2 = ct([128, 1], tag="pm2")
    nc.vector.reduce_max(pm2[:], lg[:], axis=AX.X)
    m2 = ct([128, 1], tag="m2")
    nc.gpsimd.partition_all_reduce(m2[:], pm2[:], channels=128,
                                   reduce_op=bass_isa.ReduceOp.max)
    negm2 = ct([128, 1], tag="negm2")
    nc.vector.tensor_scalar_mul(negm2[:], m2[:], -1.0)
    e2 = ct([128, VT], tag="e2")
    es2 = ct([128, 1], tag="es2")
    nc.scalar.activation(e2[:], lg[:], AF.Exp, bias=negm2[:], scale=1.0,
                         accum_out=es2[:])
    s2 = ct([128, 1], tag="s2")
    nc.gpsimd.partition_all_reduce(s2[:], es2[:], channels=128,
                                   reduce_op=bass_isa.ReduceOp.add)

    ag3_row = ct([1, 2], tag="ag3_row")
    nc.scalar.copy(ag3_row[:, 0:1], m2[0:1, :])
    nc.scalar.copy(ag3_row[:, 1:2], s2[0:1, :])
    ag3_in = dp.tile([1, 2], F32, name=f"ag3_in{r}")
    ag3_out = dp.tile([NC, 2], F32, name=f"ag3_out{r}")
    nc.gpsimd.dma_start(ag3_in[:], ag3_row[:])
    nc.gpsimd.collective_compute(
        "AllGather", ALU.bypass, replica_groups=rg,
        ins=[ag3_in.opt()], outs=[ag3_out.opt()])

    r3 = ct([1, 2 * NC], tag="r3")
    nc.sync.dma_start(r3[:], ag3_out[:].rearrange("a b -> (a b)")[None, :])
    m8 = r3[:].rearrange("a (n two) -> a n two", two=2)[:, :, 0]
    s8 = r3[:].rearrange("a (n two) -> a n two", two=2)[:, :, 1]
    gM2 = ct([1, 1], tag="gM2")
    nc.vector.reduce_max(gM2[:], m8, axis=AX.X)
    d8 = ct([1, NC], tag="d8")
    nc.vector.tensor_scalar(out=d8[:], in0=m8, scalar1=gM2[:],
                            scalar2=None, op0=ALU.subtract)
    nc.scalar.activation(d8[:], d8[:], AF.Exp)
    nc.vector.tensor_tensor(out=d8[:], in0=d8[:], in1=s8, op=ALU.mult)
    S2g = ct([1, 1], tag="S2g")
    nc.vector.reduce_sum(S2g[:], d8[:], axis=AX.X)
    nc.scalar.activation(S2g[:], S2g[:], AF.Ln)
    logZ = ct([1, 1], tag="logZ")
    nc.vector.tensor_add(out=logZ[:], in0=S2g[:], in1=gM2[:])
    logZ_col = ct([128, 1], tag="logZ_col")
    nc.gpsimd.partition_broadcast(logZ_col[:], logZ[:])
    outt = ct([128, VT], tag="outt")
    nc.vector.tensor_scalar(out=outt[:], in0=lg[:], scalar1=logZ_col[:],
                            scalar2=None, op0=ALU.subtract)
    nc.sync.dma_start(out_logits[:], outt[:])


def make_in_maps(inputs):
    tok = np.asarray(inputs["input_tok"]).astype(np.int32).reshape(1, 1)
    emb = np.ascontiguousarray(np.asarray(inputs["emb"], dtype=np.float32))
    lc = np.asarray(inputs["last_context"], dtype=np.float32).reshape(1, H)
    lh = np.asarray(inputs["last_hidden"], dtype=np.float32).reshape(1, H)
    encf = np.asarray(inputs["encoder_outputs"], dtype=np.float32).reshape(S, H)
    w_ih = np.asarray(inputs["w_ih"], dtype=np.float32)
    w_hh = np.asarray(inputs["w_hh"], dtype=np.float32)
    b_ih = np.asarray(inputs["b_ih"], dtype=np.float32)
    b_hh = np.asarray(inputs["b_hh"], dtype=np.float32)
    atten_W = np.asarray(inputs["atten_W"], dtype=np.float32)
    w_score = np.asarray(inputs["w_score"], dtype=np.float32)
    out_W = np.asarray(inputs["out_W"], dtype=np.float32)
    out_b = np.asarray(inputs["out_b"], dtype=np.float32)

    wih3 = w_ih.reshape(3, H, H2)
    whh3 = w_hh.reshape(3, H, H)
    bih3 = b_ih.reshape(3, H)
    bhh3 = b_hh.reshape(3, H)
    ws_t = np.ascontiguousarray(w_score.reshape(8, 128).T)

    W8 = np.zeros((NC * VS, H2), dtype=np.float32)
    W8[:V] = out_W
    b8 = np.full(NC * VS, NEG_BIG, dtype=np.float32)
    b8[:V] = out_b

    in_maps = []
    for c in range(NC):
        hs = slice(c * 128, (c + 1) * 128)
        in_maps.append({
            "tok": tok,
            "emb": emb,
            "lc": lc,
            "lh": lh,
            "lh_col": np.ascontiguousarray(lh[0, hs].reshape(128, 1)),
            "enc": np.ascontiguousarray(encf[c * SS : (c + 1) * SS]),
            "w_ihx": np.ascontiguousarray(wih3[:, hs, :].reshape(3 * 128, H2)),
            "w_hhx": np.ascontiguousarray(whh3[:, hs, :].reshape(3 * 128, H)),
            "b_ihx": np.ascontiguousarray(bih3[:, hs].T),
            "b_hhx": np.ascontiguousarray(bhh3[:, hs].T),
            "ws_t": ws_t,
            "awx": np.ascontiguousarray(atten_W[:, H + c * 128 : H + (c + 1) * 128]),
            "outWx": np.ascontiguousarray(W8[c * VS : (c + 1) * VS]),
            "outbx": np.ascontiguousarray(
                b8[c * VS : (c + 1) * VS].reshape(VT, 128).T
            ),
        })
    return in_maps


def assemble(results):
    logits = np.concatenate(
        [results[c]["out_logits"].T.reshape(VS) for c in range(NC)]
    )[:V].reshape(1, V)
    weights = np.concatenate(
        [results[c]["out_w"].T.reshape(SS) for c in range(NC)]
    ).reshape(1, 1, S)
    context = results[0]["out_context"].reshape(1, H)
    hidden = results[0]["out_hidden"].reshape(1, 1, H)
    return logits, context, hidden, weights


_NC_CACHE = None


def kernel(**inputs):
    global _NC_CACHE
    if _NC_CACHE is None:
        _NC_CACHE = build_nc()
    in_maps = make_in_maps(inputs)
    res = run_bass_kernel_spmd(_NC_CACHE, in_maps, core_ids=list(range(NC)),
                               trace=False)
    return assemble(res.results)
